# revision 29
# baseline (speedup 1.0000x reference)
"""2-layer GCN (GCNConv -> ReLU -> GCNConv -> ReLU -> two linear heads) on 8
Trainium2 NeuronCores.

Strategy:
  - Destination nodes sharded across 8 cores (12500 each).
  - Symmetric norm deg^{-1/2} factorized: source-side factor pre-applied to
    the feature tables (host for layer 1; on-device for the layer-2 table),
    dst-side factor applied once per 128-dst block (column scale against a
    replicated deg^{-1/2} image). Aggregation indicators are then pure 0/1
    masks.
  - Aggregation as gather + mask-matmul: for each (group-of-blocks, chunk),
    one dma_gather pulls source rows (256B each, fp16) from the table in
    HBM; ONE DVE scalar_tensor_tensor per block builds all of its 0/1 mask
    tiles (iota == dstloc, via stride-0 broadcast APs); the PE multiplies
    gathered-rows^T @ mask (fp16, fp32 PSUM accumulation).
  - Gathers rotate across 4 SWDGE queues (the serial Q7 descriptor queue is
    the single-queue bottleneck; multiple queues give ~5x).
  - int16 gather indices are sign-extended by the ucode (trailing negative
    runs are trimmed - preprocess keeps the last slot of every bucket
    non-negative), so one gather covers 65536 rows with a mid-chunk base:
    layer-1 table = 2 chunks of 50000 nodes.
  - Layer 1 aggregates x~ = D^-1/2 x then applies W1; layer 2 applies W2
    locally, writes the D^-1/2-scaled p2 shard, and the shard handoff is
    TWO half-AllGathers: the first is issued as soon as the first 6250
    rows of the shard are done (overlapping the collective with layer-1's
    second half), and layer-2 chunk-0 gathers depend only on the first
    half. Layer 2's table is the pair of half-AllGather outputs ([8 cores
    x 6250 rows] each), with its own node->row mapping and gather images.
"""

import math
import os
import numpy as np

N_NODES = 100000
IN_D = 64
HID = 128
OUT_D = 64
N_CORES = 8
PER_CORE = N_NODES // N_CORES  # 12500
BLOCK = 128  # dst nodes per block (PSUM accumulator width)
P = 128
ROW = 128  # fp16 elements per padded table row (= 256B)
GROUP = int(os.environ.get("GCN_GROUP", "2"))  # blocks per dma_gather
N_CHUNKS = 2
HALF = PER_CORE // 2  # 6250: rows per half-shard in the split AllGather
CHUNK_EXT = 25000     # rows from the AP base to the chunk end

# layer 1: chunk by source id s // 50000, AP base mid-chunk
L1_BASE = [25000, 75000]


def l2_mapping(s):
    """Node id -> (chunk, row-within-half-table) for the layer-2 table:
    half h row = core*HALF + (local % HALF)."""
    c = s // PER_CORE
    r = s % PER_CORE
    h = (r >= HALF).astype(np.int64)
    row = c * HALF + (r - h * HALF)
    return h, row


# ----------------------------------------------------------------------------
# Host-side preprocessing
# ----------------------------------------------------------------------------

def _layer_images(chunk, idx_local, core, block, dstloc, nblk):
    """Bucket slots by (core, block, chunk); returns per-bucket tile budgets
    T_bc [nblk][N_CHUNKS] plus per-core idx (gather-order) and dl
    (block-major-order) images."""
    bucket = ((core * nblk) + block) * N_CHUNKS + chunk
    n_buckets = N_CORES * nblk * N_CHUNKS
    counts = np.bincount(bucket, minlength=n_buckets)
    counts3 = counts.reshape(N_CORES, nblk, N_CHUNKS)
    T_bc = np.ceil(counts3.max(axis=0) / P).astype(np.int64)
    T_bc = np.maximum(T_bc, 1)
    budgets = T_bc * P  # [nblk, chunks]
    bud_flat = budgets.reshape(-1)
    # m (mask) order: block-major
    off_bc = np.zeros(nblk * N_CHUNKS + 1, np.int64)
    np.cumsum(bud_flat, out=off_bc[1:])
    S = int(off_bc[-1])
    # g (gather) order: (group, chunk, block)
    ngrp = math.ceil(nblk / GROUP)
    g_off = np.zeros(nblk * N_CHUNKS, np.int64)
    acc = 0
    for g in range(ngrp):
        for ch in range(N_CHUNKS):
            for b in range(g * GROUP, min(nblk, (g + 1) * GROUP)):
                g_off[b * N_CHUNKS + ch] = acc
                acc += bud_flat[b * N_CHUNKS + ch]

    order = np.argsort(bucket, kind="stable")
    b_sorted = bucket[order]
    start_of = np.zeros(n_buckets + 1, np.int64)
    np.cumsum(counts, out=start_of[1:])
    pos = np.arange(len(order)) - start_of[b_sorted]
    core_of = b_sorted // (nblk * N_CHUNKS)
    local_bc = b_sorted % (nblk * N_CHUNKS)
    slot_m = core_of * S + off_bc[local_bc] + pos
    slot_g = core_of * S + g_off[local_bc] + pos

    idx_flat = np.zeros(N_CORES * S, np.int16)
    dl_flat = np.full(N_CORES * S, -1.0, np.float16)  # pads match no dst
    idx_flat[slot_g] = idx_local[order]
    dl_flat[slot_m] = dstloc[order]
    idx_flat = idx_flat.reshape(N_CORES, S)
    dl_flat = dl_flat.reshape(N_CORES, S)

    # The gather ucode trims a TRAILING run of negative indices (treats them
    # as end-of-stream), dropping their descriptors and hanging the DMA-done
    # semaphore. Keep the last slot of every bucket non-negative (pad slots
    # are idx 0, so only exactly-full buckets ever need the swap).
    for c in range(N_CORES):
        for bc in range(nblk * N_CHUNKS):
            w = int(bud_flat[bc])
            glo, mlo = int(g_off[bc]), int(off_bc[bc])
            if idx_flat[c, glo + w - 1] >= 0:
                continue
            cand = np.nonzero(idx_flat[c, glo:glo + w] >= 0)[0]
            assert len(cand), "bucket with all-negative gather indices"
            j = int(cand[0])
            a = idx_flat[c]
            a[glo + j], a[glo + w - 1] = a[glo + w - 1], a[glo + j]
            d = dl_flat[c]
            d[mlo + j], d[mlo + w - 1] = d[mlo + w - 1], d[mlo + j]

    idx_imgs = [np.ascontiguousarray(np.tile(
        idx_flat[c].reshape(-1, 16).T, (8, 1))) for c in range(N_CORES)]
    dl_imgs = [np.ascontiguousarray(
        dl_flat[c].reshape(-1, P).T) for c in range(N_CORES)]
    return [list(map(int, r)) for r in T_bc], idx_imgs, dl_imgs


def preprocess(edge_index):
    src = np.asarray(edge_index[0], dtype=np.int64)
    dst = np.asarray(edge_index[1], dtype=np.int64)

    deg = np.bincount(dst, minlength=N_NODES).astype(np.float32) + 1.0
    dinv = (1.0 / np.sqrt(deg)).astype(np.float32)

    loops = np.arange(N_NODES, dtype=np.int64)
    s_all = np.concatenate([src, loops])
    d_all = np.concatenate([dst, loops])

    core = d_all // PER_CORE
    dst_local = d_all - core * PER_CORE
    block = dst_local // BLOCK
    dstloc = (dst_local - block * BLOCK).astype(np.float16)
    nblk = math.ceil(PER_CORE / BLOCK)

    ch1 = s_all // 50000
    idx1 = (s_all - np.array(L1_BASE)[ch1]).astype(np.int16)
    T1, idx1_imgs, dl1_imgs = _layer_images(ch1, idx1, core, block, dstloc,
                                            nblk)

    ch2, row2 = l2_mapping(s_all)
    idx2 = (row2 - CHUNK_EXT).astype(np.int16)
    T2, idx2_imgs, dl2_imgs = _layer_images(ch2, idx2, core, block, dstloc,
                                            nblk)

    per_core = [{
        "idx1": idx1_imgs[c], "dl1": dl1_imgs[c],
        "idx2": idx2_imgs[c], "dl2": dl2_imgs[c],
    } for c in range(N_CORES)]
    return T1, T2, dinv, per_core


# ----------------------------------------------------------------------------
# Device program
# ----------------------------------------------------------------------------

def _tile_offsets(T_bc, nblk):
    toff = [[0] * N_CHUNKS for _ in range(nblk)]
    acc = 0
    for b in range(nblk):
        for ch in range(N_CHUNKS):
            toff[b][ch] = acc
            acc += T_bc[b][ch]
    ngrp = math.ceil(nblk / GROUP)
    gtoff = [[0] * N_CHUNKS for _ in range(nblk)]
    acc = 0
    for g in range(ngrp):
        for ch in range(N_CHUNKS):
            for b in range(g * GROUP, min(nblk, (g + 1) * GROUP)):
                gtoff[b][ch] = acc
                acc += T_bc[b][ch]
    return toff, gtoff, acc


def build_program(T1, T2, n_devices=N_CORES, collective=True, repeat=1,
                  skip_dve=False, skip_mm=False, fake_gather=False):
    import concourse.bacc as bacc
    import concourse.mybir as mybir
    import concourse.tile as tile
    from concourse.masks import make_identity
    from contextlib import ExitStack

    f32 = mybir.dt.float32
    f16 = mybir.dt.float16
    AF = mybir.ActivationFunctionType

    nblk = math.ceil(PER_CORE / BLOCK)
    ngrp = math.ceil(nblk / GROUP)
    toff1, gtoff1, nt1 = _tile_offsets(T1, nblk)
    toff2, gtoff2, nt2 = _tile_offsets(T2, nblk)

    n_queues = int(os.environ.get("GCN_QUEUES", "4"))
    nc = bacc.Bacc("TRN2", target_bir_lowering=False, debug=False,
                   num_devices=n_devices, num_swdge_queues=n_queues)

    x_d = nc.dram_tensor("x16", [N_NODES, ROW], f16, kind="ExternalInput").ap()
    idx1_d = nc.dram_tensor("idx1_img", [P, nt1 * 8], mybir.dt.int16,
                            kind="ExternalInput").ap()
    dl1_d = nc.dram_tensor("dl1_img", [P, nt1], f16,
                           kind="ExternalInput").ap()
    idx2_d = nc.dram_tensor("idx2_img", [P, nt2 * 8], mybir.dt.int16,
                            kind="ExternalInput").ap()
    dl2_d = nc.dram_tensor("dl2_img", [P, nt2], f16,
                           kind="ExternalInput").ap()
    iota_d = nc.dram_tensor("iota", [P, BLOCK], f16, kind="ExternalInput").ap()
    dvc_d = nc.dram_tensor("dinv_col", [P, nblk * BLOCK], f16,
                           kind="ExternalInput").ap()
    w1_d = nc.dram_tensor("W1", [IN_D, HID], f16, kind="ExternalInput").ap()
    b1_d = nc.dram_tensor("b1", [HID, 1], f32, kind="ExternalInput").ap()
    w2_d = nc.dram_tensor("W2", [HID, OUT_D], f16, kind="ExternalInput").ap()
    b2_d = nc.dram_tensor("b2", [OUT_D, 1], f32, kind="ExternalInput").ap()
    wh_d = nc.dram_tensor("Wh", [OUT_D, 2], f16, kind="ExternalInput").ap()
    bh_d = nc.dram_tensor("bh", [2, 1], f32, kind="ExternalInput").ap()
    out_d = nc.dram_tensor("out", [2, PER_CORE], f32, kind="ExternalOutput").ap()

    with tile.TileContext(nc) as tc, ExitStack() as es:
        consts = es.enter_context(tc.tile_pool(name="consts", bufs=1))
        dram = es.enter_context(tc.tile_pool(name="dram", bufs=1, space="DRAM"))
        p_g = es.enter_context(tc.tile_pool(
            name="p_g", bufs=int(os.environ.get("GCN_GBUFS", "4"))))
        p_ind = es.enter_context(tc.tile_pool(
            name="p_ind", bufs=int(os.environ.get("GCN_MBUFS", "3"))))
        p_sb = es.enter_context(tc.tile_pool(name="p_sb", bufs=4))
        p_out = es.enter_context(tc.tile_pool(name="p_out", bufs=3))
        p_ps_agg = es.enter_context(tc.tile_pool(name="ps_agg", bufs=4, space="PSUM"))
        p_ps_h = es.enter_context(tc.tile_pool(name="ps_h", bufs=2, space="PSUM"))
        p_ps_t = es.enter_context(tc.tile_pool(name="ps_t", bufs=1, space="PSUM"))

        iota_s = consts.tile([P, BLOCK], f16)
        nc.sync.dma_start(iota_s[:], iota_d[:])
        w1_s = consts.tile([IN_D, HID], f16)
        nc.sync.dma_start(w1_s[:], w1_d[:])
        b1_s = consts.tile([HID, 1], f32)
        nc.sync.dma_start(b1_s[:], b1_d[:])
        w2_s = consts.tile([HID, OUT_D], f16)
        nc.sync.dma_start(w2_s[:], w2_d[:])
        b2_s = consts.tile([OUT_D, 1], f32)
        nc.sync.dma_start(b2_s[:], b2_d[:])
        wh_s = consts.tile([OUT_D, 2], f16)
        nc.sync.dma_start(wh_s[:], wh_d[:])
        bh_s = consts.tile([2, 1], f32)
        nc.sync.dma_start(bh_s[:], bh_d[:])
        dvc_s = consts.tile([P, nblk * BLOCK], f16)
        nc.sync.dma_start(dvc_s[:], dvc_d[:])
        ident_s = consts.tile([P, P], f16)
        make_identity(nc, ident_s[:])
        zero_ind = consts.tile([P, BLOCK], f16)
        nc.vector.memset(zero_ind[:], 0.0)

        idx1_s = consts.tile([P, nt1 * 8], mybir.dt.int16)
        nc.sync.dma_start(idx1_s[:], idx1_d[:])
        dl1_s = consts.tile([P, nt1], f16)
        nc.sync.dma_start(dl1_s[:], dl1_d[:])
        idx2_s = consts.tile([P, nt2 * 8], mybir.dt.int16)
        nc.sync.dma_start(idx2_s[:], idx2_d[:])
        dl2_s = consts.tile([P, nt2], f16)
        nc.sync.dma_start(dl2_s[:], dl2_d[:])

        gather_ctr = [0]

        def aggregate_layer(T_bc, toff, gtoff, idx_all, dl_all, table_ap,
                            epilogue, after_block=None, ltag=""):
            for g in range(ngrp):
                blocks = list(range(g * GROUP, min(nblk, (g + 1) * GROUP)))
                g_tiles = []
                for ch in range(N_CHUNKS):
                    tsum = sum(T_bc[b][ch] for b in blocks)
                    gstart = gtoff[blocks[0]][ch]
                    ni = tsum * P
                    gt = p_g.tile([P, tsum, ROW], f16, tag=f"g{ch}{ltag}")
                    if fake_gather:
                        nc.sync.dma_start(
                            gt[:],
                            table_ap(ch)[:tsum * P, :]
                            .rearrange("(t p) e -> p t e", p=P))
                    else:
                        nc.gpsimd.dma_gather(
                            gt[:], table_ap(ch),
                            idx_all[:, gstart * 8:(gstart + tsum) * 8],
                            num_idxs=ni, num_idxs_reg=ni, elem_size=ROW,
                            single_packet=False,
                            queue_num=gather_ctr[0] % n_queues,
                        )
                    gather_ctr[0] += 1
                    g_tiles.append((gt, gstart))

                for b in blocks:
                    T_b = sum(T_bc[b])
                    # one DVE op builds all of this block's 0/1 mask tiles:
                    # ind[p, t, d] = (iota[d] == dl[p, toff_b + t])
                    if skip_dve:
                        ind_big = None
                    else:
                        t0 = toff[b][0]
                        ind_big = p_ind.tile([P, T_b, BLOCK], f16, tag="ind")
                        nc.vector.scalar_tensor_tensor(
                            ind_big[:],
                            iota_s[:].unsqueeze(1)
                            .broadcast_to([P, T_b, BLOCK]),
                            1.0,
                            dl_all[:, t0:t0 + T_b].unsqueeze(2)
                            .broadcast_to([P, T_b, BLOCK]),
                            op0=mybir.AluOpType.mult,
                            op1=mybir.AluOpType.is_equal,
                        )

                    psum = p_ps_agg.tile([IN_D, BLOCK], f32, tag="agg")
                    n_mm = T_b
                    k = 0
                    for ch in range(N_CHUNKS):
                        gt, gstart = g_tiles[ch]
                        sub = gtoff[b][ch] - gstart
                        for t in range(T_bc[b][ch]):
                            col = toff[b][ch] + t - toff[b][0]
                            k += 1
                            if skip_mm and 1 < k < n_mm:
                                continue
                            ind = (zero_ind[:] if skip_dve
                                   else ind_big[:, col, :])
                            nc.tensor.matmul(
                                psum[:], gt[:, sub + t, :IN_D],
                                ind, start=(k == 1), stop=(k == n_mm),
                            )
                    epilogue(b, psum)
                    if after_block is not None:
                        after_block(b)

        for _rep in range(repeat):
            p2_loc = [dram.tile([HALF, OUT_D], f16, name=f"p2loc{_rep}_{h}")
                      for h in range(2)]
            if collective:
                p2_half = [dram.tile([N_CORES * HALF, OUT_D], f16,
                                     addr_space="Shared",
                                     name=f"p2half{_rep}_{h}")
                           for h in range(2)]
            else:
                p2_half = [dram.tile([N_CORES * HALF, OUT_D], f16,
                                     name=f"p2half{_rep}_{h}")
                           for h in range(2)]
            # gather tables need 256B row stride: expand compact AG output
            p2_tab = [dram.tile([N_CORES * HALF, ROW], f16,
                                name=f"p2tab{_rep}_{h}") for h in range(2)]

            # ---------------- layer 1 ----------------
            def epi1(b, psum):
                cs = dvc_s[:IN_D, b * BLOCK:(b + 1) * BLOCK]
                aggT = p_sb.tile([IN_D, BLOCK], f16, tag="aggT")
                nc.vector.tensor_tensor(aggT[:], psum[:], cs,
                                        op=mybir.AluOpType.mult)
                ps_h = p_ps_h.tile([HID, BLOCK], f32, tag="mm")
                nc.tensor.matmul(ps_h[:], w1_s[:], aggT[:], start=True,
                                 stop=True)
                h1 = p_sb.tile([HID, BLOCK], f16, tag="h1")
                nc.scalar.activation(h1[:], ps_h[:], AF.Relu, bias=b1_s[:, :1])
                ps_p_full = p_ps_h.tile([HID, BLOCK], f32, tag="mm")
                ps_p = ps_p_full[:OUT_D]
                nc.tensor.matmul(ps_p, w2_s[:], h1[:], start=True, stop=True)
                p2T = p_sb.tile([OUT_D, BLOCK], f16, tag="p2T")
                nc.vector.tensor_tensor(
                    p2T[:], ps_p, dvc_s[:OUT_D, b * BLOCK:(b + 1) * BLOCK],
                    op=mybir.AluOpType.mult)
                ps_t_full = p_ps_t.tile([P, max(BLOCK, OUT_D)], f16,
                                        tag="smallh")
                ps_t = ps_t_full[:BLOCK, :OUT_D]
                nc.tensor.transpose(ps_t, p2T[:, :],
                                    identity=ident_s[:OUT_D, :OUT_D])
                p2s = p_sb.tile([BLOCK, OUT_D], f16, tag="p2s")
                nc.scalar.activation(p2s[:], ps_t, AF.Copy)
                rows0 = b * BLOCK
                nrows = min(BLOCK, PER_CORE - rows0)
                # split the store across the two half-shards
                lo_rows = min(max(HALF - rows0, 0), nrows)
                if lo_rows > 0:
                    nc.sync.dma_start(
                        p2_loc[0][rows0:rows0 + lo_rows, :OUT_D],
                        p2s[:lo_rows, :])
                if nrows - lo_rows > 0:
                    r1 = rows0 + lo_rows - HALF
                    nc.sync.dma_start(
                        p2_loc[1][r1:r1 + nrows - lo_rows, :OUT_D],
                        p2s[lo_rows:nrows, :])

            ag_blocks = {}

            def emit_ag(h):
                if collective:
                    nc.gpsimd.collective_compute(
                        "AllGather", mybir.AluOpType.bypass,
                        ins=[p2_loc[h].opt()], outs=[p2_half[h].opt()],
                        replica_groups=[list(range(N_CORES))],
                    )
                else:
                    nc.sync.dma_start(p2_half[h][:HALF, :].opt(),
                                      p2_loc[h][:].opt())
                nc.sync.dma_start(p2_tab[h][:, :OUT_D], p2_half[h][:])

            # first half-shard complete after the block covering row HALF-1
            b_half = (HALF - 1) // BLOCK

            def after_block1(b):
                if b == b_half:
                    emit_ag(0)
                elif b == nblk - 1:
                    emit_ag(1)

            aggregate_layer(T1, toff1, gtoff1, idx1_s, dl1_s,
                            lambda ch: x_d[L1_BASE[ch]:L1_BASE[ch] + CHUNK_EXT, :],
                            epi1, after_block1, ltag="a")

            # ---------------- layer 2 + heads ----------------
            def epi2(b, psum):
                h2p = p_sb.tile([OUT_D, BLOCK], f32, tag="h2p")
                nc.vector.tensor_tensor(
                    h2p[:], psum[:], dvc_s[:OUT_D, b * BLOCK:(b + 1) * BLOCK],
                    op=mybir.AluOpType.mult)
                h2 = p_sb.tile([OUT_D, BLOCK], f16, tag="h2")
                nc.scalar.activation(h2[:], h2p[:], AF.Relu, bias=b2_s[:, :1])
                ps_o_full = p_ps_t.tile([P, max(BLOCK, OUT_D)], f32,
                                        tag="small")
                ps_o = ps_o_full[:2, :BLOCK]
                nc.tensor.matmul(ps_o, wh_s[:], h2[:], start=True, stop=True)
                ob = p_out.tile([2, BLOCK], f32, tag="ob")
                nc.vector.tensor_scalar_add(ob[:], ps_o, bh_s[:, :1])
                ncols = min(BLOCK, PER_CORE - b * BLOCK)
                nc.sync.dma_start(out_d[:, b * BLOCK:b * BLOCK + ncols],
                                  ob[:, :ncols])

            aggregate_layer(T2, toff2, gtoff2, idx2_s, dl2_s,
                            lambda ch: p2_tab[ch].opt()[CHUNK_EXT:2 * CHUNK_EXT, :],
                            epi2, ltag="b")

    nc.compile()
    return nc


# ----------------------------------------------------------------------------
# Entry point
# ----------------------------------------------------------------------------

def make_in_maps(inputs, dinv, per_core):
    x = np.asarray(inputs["x"], dtype=np.float32)
    x16 = np.zeros((N_NODES, ROW), np.float16)
    x16[:, :IN_D] = (x * dinv[:, None]).astype(np.float16)
    iota = np.broadcast_to(np.arange(BLOCK, dtype=np.float16), (P, BLOCK))
    wh = np.concatenate([np.asarray(inputs["Wd"], np.float32),
                         np.asarray(inputs["Wp"], np.float32)], axis=1)
    bh = np.array([[np.float32(np.asarray(inputs["bd"]).reshape(-1)[0])],
                   [np.float32(np.asarray(inputs["bp"]).reshape(-1)[0])]],
                  np.float32)
    in_maps = []
    for c in range(N_CORES):
        npad = math.ceil(PER_CORE / BLOCK) * BLOCK
        dv = np.ones(npad, np.float16)
        dv[:PER_CORE] = dinv[c * PER_CORE:(c + 1) * PER_CORE].astype(np.float16)
        dvc = np.broadcast_to(dv, (P, npad))
        in_maps.append({
            "x16": x16,
            "idx1_img": per_core[c]["idx1"],
            "dl1_img": per_core[c]["dl1"],
            "idx2_img": per_core[c]["idx2"],
            "dl2_img": per_core[c]["dl2"],
            "iota": np.ascontiguousarray(iota),
            "dinv_col": np.ascontiguousarray(dvc),
            "W1": np.asarray(inputs["W1"], np.float32).astype(np.float16),
            "b1": np.asarray(inputs["b1"], np.float32).reshape(HID, 1),
            "W2": np.asarray(inputs["W2"], np.float32).astype(np.float16),
            "b2": np.asarray(inputs["b2"], np.float32).reshape(OUT_D, 1),
            "Wh": np.ascontiguousarray(wh).astype(np.float16),
            "bh": bh,
        })
    return in_maps


def kernel(x, edge_index, W1, b1, W2, b2, Wd, bd, Wp, bp):
    from concourse import bass_utils

    T1, T2, dinv, per_core = preprocess(edge_index)
    nc = build_program(T1, T2)
    in_maps = make_in_maps(dict(x=x, W1=W1, b1=b1, W2=W2, b2=b2, Wd=Wd,
                                bd=bd, Wp=Wp, bp=bp), dinv, per_core)
    res = bass_utils.run_bass_kernel_spmd(nc, in_maps,
                                          core_ids=list(range(N_CORES)))
    dur = np.empty((N_NODES, 1), np.float32)
    pha = np.empty((N_NODES, 1), np.float32)
    for c in range(N_CORES):
        o = res.results[c]["out"]
        dur[c * PER_CORE:(c + 1) * PER_CORE, 0] = o[0]
        pha[c * PER_CORE:(c + 1) * PER_CORE, 0] = o[1]
    return dur, pha
